# revision 55
# baseline (speedup 1.0000x reference)
"""LSH similarity-matrix kernel for Trainium2 (8 NeuronCores, data-parallel
over batch).

Math: reference computes, per (l, b):
    c1 = (query_embed @ r.T > 0),  c2 = (doc_embed @ r.T > 0)   in {0,1}
    ham = s1 + s2 - 2*c1@c2.T ;  sim = cos(pi/NB * ham), masked where tok==0.
With +-1 codes U = 2c-1 and S = U1 @ U2.T:  ham = (NB - S)/2, so
    sim = sin(pi/(2*NB) * S).
Masks fold into the embeddings: a zeroed embedding row projects to 0,
sign(0) = 0 gives a zero code row, so S = 0 and sin(0) = 0 — exactly the
masked output. Masked doc tokens (half: tok in {0,1}) are gathered away
host-side; output scatters back as zeros. Batches are assigned to
(core, slot) sorted by active-doc count so every core runs an identically
shaped SPMD program with minimal padding.

Precision: embeddings and the projection matrix ship as fp16 (11-bit
mantissa = TF32-class hash-bit flips; fp16 x fp16 products are exact in
the f32 PSUM accumulator). r is pre-scaled by 2^14 (fp16-range-safe) so
the DVE sign alternative clamp(x,-1,1) is +-1 for any |proj| >= 2^-14;
the ~4e-5 fraction of bits below that threshold round to ~0 in fp8 — a
half-weight error on ~3k of 71M bits. Measured end-to-end rel err 9.4e-3
(gate 2e-2).

Layout: per slot the active docs split into a MAIN window (first <=512,
SPMD-padded with zero rows) and a tiny RESIDUAL (docs 512..p; p<=560
here). MAIN projects as one 512-col fp16 matmul per (layer, chunk), two
chunks to a 2-bank PSUM tile (3 rotating tiles = 6 banks) so each ACT/DVE
sign op drains 1024 columns, amortizing the ~185/125 ns per-op init; the
first two tiles of the first slot are single-chunk so the drain engines
start ~1 us earlier. RESIDUAL (both layers, all 8 chunks) packs into one
1-bank tile: fp16 moving operands have no small-size penalty (unlike
f32r's 4x below 256 cols). Sign work is split between ACT (Sign) and DVE
(clamp) by an arrival-aware makespan balancer (_BAL_PE_NS/_BAL_BIAS tuned
against TimelineSim); these two engines are the only PSUM drains on TRN2
and are the kernel's bottleneck (PE ~58% busy).

The code dot runs transposed — S^T[d, q] — with docs on the PSUM
partition dim in groups of 128: stationary = doc-code chunk pairs (fp8
DoubleRow, K=256/matmul), moving = the slot's query codes (qpad=40 cols,
~8 ns each). Both layers and all 4-5 groups of a slot accumulate into ONE
psum bank [128, 2*G*qpad], so the sin is a single 320-400 column ACT op
per slot (vs ~520 cols per layer-job with queries on partitions) and the
output is one DMA per slot; garbage partitions of the partial group cost
nothing (engine time = free-dim size) and are never read back. The last
slot (smallest) splits stage-C per layer to shorten the serial tail. A
dummy Sin+Sign at t=0 hoists the 1.3 us activation-table load into the
DMA window; dependency-free bf16 warmup matmuls carry the PE through its
p-state clock ramp.

Output: fp16 (sim in [-1,1]; 5e-4 relative rounding), cast f32 host-side.
TimelineSim: 31.4 us/core vs 34.2 us for the query-major baseline.
"""
import os
import sys

sys.path.insert(0, "/opt/trn_rl_repo")

from contextlib import ExitStack

import numpy as np

import concourse.bass as bass
import concourse.mybir as mybir
import concourse.tile as tile
from concourse import bacc
from concourse.bass_utils import run_bass_kernel_spmd

L, BAT, A, BDOC, D, NB = 2, 32, 64, 1024, 128, 1024
CORES = 8
BPC = BAT // CORES          # batch slots per core
CH = NB // 128              # 8 bit-chunks
MAIN = 512                  # main doc window per (slot, layer)
SCALE = float(2.0 ** 14)
PI = float(np.pi)
N_WARM = 4

F32 = mybir.dt.float32
F32R = mybir.dt.float32r
F16 = mybir.dt.float16
BF16 = mybir.dt.bfloat16
FP8 = mybir.dt.float8e4
Alu = mybir.AluOpType
Act = mybir.ActivationFunctionType
DR = mybir.MatmulPerfMode.DoubleRow

_BUILD_CACHE: dict = {}

# v2 cost-model constants for the ACT/DVE makespan balancer (ns)
_ACT_NS = 1e9 / 1.2e9
_DVE_NS = 1e9 / 0.96e9
_ACT_INIT = 370.0           # 2*222 SBUF-out cycles @0.833 (busy+ack)
_DVE_INIT = 250.0           # 2*120 PSUM-in cycles @1.0417

_BAL_BIAS = [-150.0]
_BAL_PE_NS = [0.64]          # effective PE ns/col for arrival modeling
# sim-tuned endgame flips (hill-climbed against TimelineSim): the final
# single-chunk sign of the last slot and one late residual op move to DVE
_BAL_OVERRIDE: dict = {("d", 0, 1, 4): "v", ("r", 1, 0, 1): "v"}


def _slot_meta(pads_c):
    """Per-slot (main_width, residual_width, n_groups)."""
    meta = []
    for p in pads_c:
        m = min(p, MAIN)
        r = p - m
        g = (p + 127) // 128
        meta.append((m, r, g))
    return meta


def _balance(pads_c, sorder, qpad):
    """Assign sign ops to ACT ('a') / DVE ('v') with an arrival-aware
    online greedy over the emission sequence: each op becomes available
    when the PE finishes its PSUM tile (a running producer clock) and
    goes to the engine that finishes it first. ACT additionally absorbs
    each slot's Sin at its stage-C position."""
    meta = _slot_meta(pads_c)
    qw = BPC * L * qpad
    act = 2 * 198.0 + 1283.0      # dummies + LoadActFuncSet
    dve = 0.0
    pe = 2500.0                   # first projection tile completes ~here
    bias = _BAL_BIAS[0]
    assign = {}

    pe_ns = _BAL_PE_NS[0]

    def put(key, n):
        nonlocal act, dve, pe
        pe += n * pe_ns
        ca = n * _ACT_NS + _ACT_INIT / 2
        cv = n * _DVE_NS + _DVE_INIT / 2
        if max(act, pe) + ca + bias <= max(dve, pe) + cv:
            act = max(act, pe) + ca
            assign[key] = "a"
        else:
            dve = max(dve, pe) + cv
            assign[key] = "v"

    def put_proj(s, granular=False, endgran=False):
        m, r, _g = meta[s]
        for l in range(L):
            if granular and l == 0:
                widths = [m, m, 2 * m, 2 * m, 2 * m]
            elif endgran and l == L - 1:
                widths = [2 * m, 2 * m, 2 * m, m, m]
            else:
                widths = [2 * m] * 4
            for t, w in enumerate(widths):
                put(("d", s, l, t), w)
        if r > 0:
            nt = 1 if r <= 32 else 2
            for t in range(nt):
                for l in range(L):
                    put(("r", s, t, l), (CH // nt) * r)

    def put_sin(s):
        nonlocal act, pe
        _m, _r, g = meta[s]
        n = 2 * g * qpad
        pe += n * 0.21
        act = max(act, pe) + n * _ACT_NS + _ACT_INIT / 2

    # emission mirrors _build: query tiles early, then slot projs with
    # previous slot's stage-C between them
    put_proj(sorder[0], granular=True)
    for t in range(4):
        put(("q", t), 2 * qw)
    put_proj(sorder[1])
    for i in range(BPC):
        if i + 2 < BPC:
            put_proj(sorder[i + 2], endgran=(i + 2 == BPC - 1))
        put_sin(sorder[i])
    _balance.totals = (act, dve)
    for k, v in _BAL_OVERRIDE.items():
        if k in assign:
            assign[k] = v
    return assign


def _build(pads_c: tuple, qpad: int = 40, reps: int = 1):
    """Per-core SPMD program. pads_c[s]: padded doc count (multiple of 8)
    of batch slot s, shared by both layers. reps > 1 re-emits the whole
    body (timing instrumentation only)."""
    pads_c = tuple(int(p) for p in pads_c)
    meta = _slot_meta(pads_c)
    rsum = sum(r for _m, r, _g in meta)
    gmax = max(g for _m, _r, g in meta)
    qw = BPC * L * qpad
    # largest slots first; smallest last so the tail drains quickly
    sorder = sorted(range(BPC), key=lambda s: -pads_c[s])
    assign = _balance(pads_c, sorder, qpad)

    nc = bacc.Bacc("TRN2", target_bir_lowering=False, debug=False)

    pmax = max(pads_c)
    QE = nc.dram_tensor("qe", [D, qw], F16, kind="ExternalInput").ap()
    DE = nc.dram_tensor("de", [BPC, D, L * pmax], F16,
                        kind="ExternalInput").ap()
    RT = nc.dram_tensor("rt", [D, NB], F16, kind="ExternalInput").ap()
    # per-partition rows are packed [l][g][q] CONTIGUOUSLY (2*g*qpad
    # cols, slot-dependent g) so out-DMA runs are 640-800B — below 512B
    # the DMA bus charges a 2x read-modify-write penalty and one
    # descriptor per 80B run
    OUT = nc.dram_tensor("out", [BPC, 128, L * gmax * qpad], F16,
                         kind="ExternalOutput").ap()

    with tile.TileContext(nc) as tc, ExitStack() as ctx:
        const = ctx.enter_context(tc.tile_pool(name="const", bufs=1))
        jobp = ctx.enter_context(tc.tile_pool(name="jobp", bufs=4))
        outp = ctx.enter_context(tc.tile_pool(name="outp", bufs=4))
        # PSUM: proj pool 3 x 2-bank + shared pool (S / residual) 2 x 1-bank
        ps_p = ctx.enter_context(tc.tile_pool(name="ps_p", bufs=3,
                                              space="PSUM"))
        ps_s = ctx.enter_context(tc.tile_pool(name="ps_s", bufs=2,
                                              space="PSUM"))

        for _rep in range(reps):
            _rp = f"r{_rep}_"
            rt = const.tile([D, NB], F16, tag="rt", name=f"{_rp}rt")
            qe = const.tile([D, qw], F16, tag="qe", name=f"{_rp}qe")
            U1 = const.tile([D, CH * qw], FP8, tag="U1", name=f"{_rp}U1")
            warm = const.tile([D, 512], BF16, tag="warm", name=f"{_rp}warm")
            wsin = const.tile([D, 16], F16, tag="wsin", name=f"{_rp}wsin")

            det = {}
            U2 = {}
            U2R = {}

            def dma_de(s):
                det[s] = jobp.tile([D, L * pmax], F16, tag="det",
                                   name=f"{_rp}det{s}")
                U2[s] = jobp.tile([D, L * CH * MAIN], FP8, tag="U2",
                                  name=f"{_rp}U2{s}")
                p = pads_c[s]
                # host stages [l][p] packed at stride p
                nc.sync.dma_start(out=det[s][:, 0:L * p],
                                  in_=DE[s, :, 0:L * p])

            # ---- startup: lead the DMA queue with what the first compute
            # needs. de of the first slot split per layer so projection of
            # layer 0 starts as early as the DGE latency allows ----
            s0 = sorder[0]
            p0 = pads_c[s0]
            det[s0] = jobp.tile([D, L * pmax], F16, tag="det",
                                name=f"{_rp}det{s0}")
            U2[s0] = jobp.tile([D, L * CH * MAIN], FP8, tag="U2",
                               name=f"{_rp}U2{s0}")
            nc.sync.dma_start(out=rt[:, 0:256], in_=RT[:, 0:256])
            nc.sync.dma_start(out=det[s0][:, 0:p0], in_=DE[s0, :, 0:p0])
            nc.sync.dma_start(out=rt[:, 256:NB], in_=RT[:, 256:NB])
            nc.sync.dma_start(out=det[s0][:, p0:L * p0],
                              in_=DE[s0, :, p0:L * p0])
            nc.sync.dma_start(out=qe, in_=QE)
            dma_de(sorder[1])

            # dummy Sin then Sign hoist the single LoadActFuncSet
            # (trig_and_small holds both) into the DMA window; dummy
            # matmuls pull the PE through its p-state ramp while the
            # first DMAs land
            nc.gpsimd.memset(warm, 0.0)
            nc.scalar.activation(wsin, warm[:, 0:16], Act.Sin, scale=1.0)
            nc.scalar.activation(wsin, warm[:, 0:16], Act.Sign)
            wps = ps_s.tile([D, 512], F32, tag="ss", name=f"{_rp}wps")
            for _ in range(N_WARM):
                nc.tensor.matmul(wps, warm[:, 0:128], warm,
                                 start=True, stop=True)

            def sign_op(key, out_ap, in_ap):
                if assign[key] == "a":
                    nc.scalar.activation(out_ap, in_ap, Act.Sign)
                else:
                    nc.vector.tensor_scalar(out_ap, in_ap, 1.0, -1.0,
                                            Alu.min, Alu.max)

            def proj_tile(s, l, t, k0, nk, halves=False):
                """One projection tile of slot s, layer l covering chunks
                k0..k0+nk (nk=1 single-chunk starters, nk=2 steady).
                halves: emit 256-col half matmuls so the first tiles fire
                on the earlier half-DMA completion semaphore."""
                m = meta[s][0]
                p = pads_c[s]
                ps = ps_p.tile([D, 1024], F32, tag="pp",
                               name=f"{_rp}pp{s}_{l}_{t}")
                for i in range(nk):
                    if halves:
                        for c0 in range(0, m, 256):
                            cw = min(256, m - c0)
                            nc.tensor.matmul(
                                ps[:, i * 512 + c0:i * 512 + c0 + cw],
                                rt[:, (k0 + i) * 128:(k0 + i + 1) * 128],
                                det[s][:, l * p + c0:l * p + c0 + cw],
                                start=True, stop=True)
                    else:
                        nc.tensor.matmul(
                            ps[:, i * 512:i * 512 + m],
                            rt[:, (k0 + i) * 128:(k0 + i + 1) * 128],
                            det[s][:, l * p:l * p + m],
                            start=True, stop=True)
                iv = ps[:].rearrange("q (a x) -> q a x",
                                     x=512)[:, 0:nk, 0:m]
                ov = U2[s][:, (l * CH + k0) * MAIN:
                           (l * CH + k0 + nk) * MAIN] \
                    .rearrange("q (a x) -> q a x", x=MAIN)[:, :, 0:m]
                sign_op(("d", s, l, t), ov, iv)

            def res_tiles(s):
                """Residual projection of slot s (both layers, fp16
                moving straight from det — no small-operand penalty)."""
                r = meta[s][1]
                if r == 0:
                    return
                U2R[s] = jobp.tile([D, L * CH * 64], FP8, tag="U2R",
                                   name=f"{_rp}U2R{s}")
                u2r = U2R[s]
                nt = 1 if r <= 32 else 2        # 1-bank tiles of 8 or 4 ch
                cw = 64 if r <= 32 else 128
                for t in range(nt):
                    kpt = CH // nt
                    ps = ps_s.tile([D, 512], F32, tag="ss",
                                   name=f"{_rp}pr{s}_{t}")
                    for i in range(kpt):
                        k = t * kpt + i
                        nc.tensor.matmul(
                            ps[:, i * cw:(i + 1) * cw]
                            .rearrange("q (a x) -> q a x",
                                       x=cw // 2)[:, 0:2, 0:r],
                            rt[:, k * 128:(k + 1) * 128],
                            det[s][:, 0:L * pads_c[s]]
                            .rearrange("q (l x) -> q l x", l=L)
                            [:, :, meta[s][0]:meta[s][0] + r],
                            start=True, stop=True)
                    for l in range(L):
                        iv = ps[:].rearrange("q (a x) -> q a x",
                                             x=cw)[:, 0:kpt,
                                                   l * (cw // 2):
                                                   l * (cw // 2) + r]
                        ov = u2r[:, l * CH * 64 + t * kpt * 64:
                                 l * CH * 64 + (t + 1) * kpt * 64] \
                            .rearrange("q (a x) -> q a x",
                                       x=64)[:, :, 0:r]
                        sign_op(("r", s, t, l), ov, iv)

            def query_tile(t):
                qp = ps_p.tile([D, 1024], F32, tag="pp",
                               name=f"{_rp}qp{t}")
                for i in range(2):
                    nc.tensor.matmul(qp[:, i * 512:i * 512 + qw],
                                     rt[:, (2 * t + i) * 128:
                                        (2 * t + i + 1) * 128], qe,
                                     start=True, stop=True)
                iv = qp[:].rearrange("q (a x) -> q a x",
                                     x=512)[:, 0:2, 0:qw]
                ov = U1[:, 2 * t * qw:(2 * t + 2) * qw] \
                    .rearrange("q (a x) -> q a x", x=qw)
                sign_op(("q", t), ov, iv)

            def dot_burst(s, S, l):
                """Code dots of one layer of slot s (fp8 DoubleRow, docs
                on partitions, groups of 128) into the shared S tile."""
                m, r, g = meta[s]
                u2 = U2[s]
                qcol = (s * L + l) * qpad
                for gi in range(g):
                    oc = (l * g + gi) * qpad
                    for jj in range(CH // 2):
                        if gi * 128 < m:
                            nd = min(128, m - gi * 128)
                            lw = u2[:].rearrange(
                                "q (a x) -> q a x", x=MAIN) \
                                [:, l * CH + 2 * jj:l * CH + 2 * jj + 2,
                                 gi * 128:gi * 128 + nd]
                        else:
                            lw = U2R[s][:].rearrange(
                                "q (a x) -> q a x", x=64) \
                                [:, l * CH + 2 * jj:l * CH + 2 * jj + 2,
                                 0:r]
                            nd = r
                        rv = U1[:].rearrange("q (a x) -> q a x", x=qw) \
                            [:, 2 * jj:2 * jj + 2, qcol:qcol + qpad]
                        nc.tensor.matmul(
                            S[0:nd, oc:oc + qpad], lw, rv,
                            start=(jj == 0), stop=(jj == CH // 2 - 1),
                            perf_mode=DR)

            def sin_dma(s, S, l=None, swdge=False):
                """Fused sin + output DMA; l=None covers both layers.
                swdge routes the DMA through the idle Pool engine's
                software DGE (tail only)."""
                g = meta[s][2]
                lr = range(L) if l is None else (l,)
                n = len(lr) * g * qpad
                sim = outp.tile([128, L * gmax * qpad], F16, tag="sim",
                                name=f"{_rp}sim{s}_{l}")[:, 0:n]
                c0 = (0 if l is None else l) * g * qpad
                nc.scalar.activation(sim, S[:, c0:c0 + n], Act.Sin,
                                     scale=PI / (2.0 * NB))
                # sbuf [128d, (l, g, q)] -> dram contiguous row slice
                nc.sync.dma_start(out=OUT[s, :, c0:c0 + n], in_=sim)

            def stage_c_units(s, split_tail=False):
                """Stage-C emission units for slot s: dot bursts and the
                fused sin + output DMA (per layer when split_tail)."""
                g = meta[s][2]
                S = ps_s.tile([D, 2 * g * qpad], F32, tag="ss",
                              name=f"{_rp}S{s}")
                if split_tail:
                    yield lambda: dot_burst(s, S, 0)
                    yield lambda: sin_dma(s, S, 0)
                    yield lambda: dot_burst(s, S, 1)
                    yield lambda: sin_dma(s, S, 1)
                else:
                    yield lambda: dot_burst(s, S, 0)
                    yield lambda: dot_burst(s, S, 1)
                    yield lambda: sin_dma(s, S)

            def stage_b_units(s, granular=False, endgran=False):
                for l in range(L):
                    if granular and l == 0:
                        plan = [(0, 1), (1, 1), (2, 2), (4, 2), (6, 2)]
                    elif endgran and l == L - 1:
                        plan = [(0, 2), (2, 2), (4, 2), (6, 1), (7, 1)]
                    else:
                        plan = [(2 * t, 2) for t in range(4)]
                    for t, (k0, nk) in enumerate(plan):
                        yield lambda l=l, t=t, k0=k0, nk=nk: \
                            proj_tile(s, l, t, k0, nk)
                yield lambda: res_tiles(s)

            def interleave(b_units, c_units, b_first=False):
                """Round-robin: ~3 proj tiles per stage-C unit so dots
                slot into PE stalls between drain-limited tiles. b_first
                front-loads all proj tiles (endgame: the last slot's
                signs must clear the drain queues as early as possible)."""
                b_units = list(b_units)
                c_units = list(c_units)
                ci = 0
                for i, u in enumerate(b_units):
                    u()
                    if not b_first and i % 3 == 2 and ci < len(c_units):
                        c_units[ci]()
                        ci += 1
                while ci < len(c_units):
                    c_units[ci]()
                    ci += 1

            # ---- emission: proj of slot i+2 interleaved with stage-C of
            # slot i so the in-order PE queue never parks a dot behind
            # signs it would stall on ----
            for u in stage_b_units(sorder[0], granular=True):
                u()
            if BPC > 2:
                dma_de(sorder[2])
            for t in range(4):
                query_tile(t)
            interleave(stage_b_units(sorder[1]), [])
            for i in range(BPC):
                if i + 3 < BPC:
                    dma_de(sorder[i + 3])
                b = (stage_b_units(sorder[i + 2],
                                   endgran=(i + 2 == BPC - 1))
                     if i + 2 < BPC else [])
                interleave(b, stage_c_units(sorder[i],
                                            split_tail=(i == BPC - 1)),
                           b_first=(i == BPC - 3))

    nc.compile()
    return nc


def _stage_inputs(query_embed, doc_embed, query_tok, doc_tok, r):
    query_embed = np.ascontiguousarray(query_embed, dtype=np.float32)
    doc_embed = np.ascontiguousarray(doc_embed, dtype=np.float32)
    r = np.ascontiguousarray(r, dtype=np.float32)

    qmask = (np.asarray(query_tok) != 0)
    dmask = (np.asarray(doc_tok) != 0)

    # sort batches by active doc count; slot s takes ranks [s*CORES,
    # (s+1)*CORES) spread across the 8 cores, so per-slot padding is tight
    # and identical on every core (SPMD: one shape per slot)
    counts = dmask.sum(axis=1).astype(int)
    order = np.argsort(counts, kind="stable")
    assign = np.empty((CORES, BPC), dtype=int)   # assign[c, s] = batch id
    for s in range(BPC):
        for c in range(CORES):
            assign[c, s] = order[s * CORES + c]
    pads_c = tuple(
        min(BDOC, max(288, int(-(-int(counts[assign[:, s]].max()) // 8) * 8)))
        for s in range(BPC)
    )
    meta = _slot_meta(pads_c)
    gmax = max(g for _m, _r, g in meta)

    qe_m = query_embed * qmask[None, :, :, None].astype(np.float32)
    rts = np.ascontiguousarray((r.T * SCALE).astype(np.float16))

    qidxs = [np.flatnonzero(qmask[g]) for g in range(BAT)]
    qpad = min(A, max(32, max(len(q) for q in qidxs)))

    pmax = max(pads_c)
    idxs = [np.flatnonzero(dmask[g]) for g in range(BAT)]
    in_maps = []
    for c in range(CORES):
        qe_c = np.zeros((D, BPC * L * qpad), dtype=np.float16)
        de_c = np.zeros((BPC, D, L * pmax), dtype=np.float16)
        for s in range(BPC):
            g = assign[c, s]
            p = pads_c[s]
            idx = idxs[g]
            qi = qidxs[g]
            for li in range(L):
                qe_c[:, (s * L + li) * qpad:(s * L + li) * qpad + len(qi)] \
                    = qe_m[li, g, qi].T.astype(np.float16)
                de_c[s, :, li * p:li * p + len(idx)] = \
                    doc_embed[li, g, idx].T.astype(np.float16)
        in_maps.append({"qe": qe_c, "de": de_c, "rt": rts})

    return in_maps, assign, idxs, pads_c, qidxs, qpad


def kernel(query_embed, doc_embed, query_tok, doc_tok, r):
    in_maps, assign, idxs, pads_c, qidxs, qpad = _stage_inputs(
        query_embed, doc_embed, query_tok, doc_tok, r)

    key = (pads_c, qpad)
    if key not in _BUILD_CACHE:
        _BUILD_CACHE[key] = _build(pads_c, qpad)
    nc = _BUILD_CACHE[key]

    res = run_bass_kernel_spmd(nc, in_maps, core_ids=list(range(CORES)))

    meta = _slot_meta(pads_c)
    out = np.zeros((BAT, L, A, BDOC), dtype=np.float32)
    for c in range(CORES):
        o_c = np.asarray(res.results[c]["out"]).astype(np.float32)
        for s in range(BPC):
            g = assign[c, s]
            _m, _r, gg = meta[s]
            idx = idxs[g]
            qi = qidxs[g]
            for li in range(L):
                # o_c[s]: [128, L*gmax*qpad] rows packed [l][g_s][q]
                row = o_c[s][:, li * gg * qpad:(li + 1) * gg * qpad]
                flat = row.reshape(128, gg, qpad).transpose(1, 0, 2) \
                    .reshape(-1, qpad)
                out[g, li][np.ix_(qi, idx)] = flat[:len(idx), :len(qi)].T
    return out
